# revision 69
# baseline (speedup 1.0000x reference)
"""Multi-head causal attention (B=2, T=2048, D=1024, H=16) on 8 TRN2 cores.

Sharding: core c handles batch b=c//4 and head group g=c%4 (4 heads each).

Device kernel per core:
  Projections (Q/K/V) run as fp8 DoubleRow matmuls (256-deep contraction,
  0.5 PE cycles per output column) with full precision recovered via a
  hi/lo split prepared on the host: X = Xh(e4m3) + Xl(e5m2 residual),
  W likewise, and X@W = XhWh + XhWl + XlWh (XlWl ~ 0.1% dropped; e5m2 for
  the lo plane because the residuals underflow e4m3 denormals).
  Attention per q-tile (512 q) per head pair, two phases:
    A: per key-block kb (128 k): ST[k, 2*q] = KT-vs-QT f16 matmul (2 heads
       row-packed on the PE array), PT = exp(ST/8) f16 kept in SBUF (one
       tile per (hp, kb)), diag blocks masked via DVE mul.
    B: transposed PV: per (head, 128-q chunk): CT_t[q, 65] += PT-chunk-vs-
       V_aug matmul over kb (PE cost 65 instead of 128+ cycles per block;
       col 64 = softmax denominator l lands per q-PARTITION). Accs packed
       4-per-PSUM-bank as sequential accumulation chains, drained by one
       copy. Normalize with per-partition 1/l (DVE tensor_scalar + one
       batched reciprocal), transpose back via PE is_transpose (identity),
       stage to ct_sb.
  OT[d, q-tile] = WO-vs-CT f16 matmul -> f16 partial, DMA'd per q-tile.
Schedule: software-pipelined rotation B(qt,0) A(qt+1,0) B(qt,1) A(qt+1,1)
with proj/out-proj chains fed into the ACT-bound A phases via a labeled
work queue (deps are emission-order).
Host: out[b] = sum over the 4 head-group cores of OT^T, + bo.
"""

import numpy as np

B, T, D, H, HD = 2, 2048, 1024, 16, 64
NCORES = 8
HPC = 4          # heads per core
GC = HPC * HD    # 256 columns per head group
NQ = 512         # q-tile width
KB = 128         # key block
QB = 128         # q chunk for transposed PV

_cache = {}


def _build(t_len):
    from concourse import bacc
    import concourse.tile as tile
    import concourse.mybir as mybir

    dt = mybir.dt
    f32, f16 = dt.float32, dt.float16

    n_qt = t_len // NQ           # q tiles (and 512-wide n tiles)
    n_tt = t_len // KB           # token tiles of 128
    n_kc = D // KB               # contraction chunks over D (8)

    nc = bacc.Bacc("TRN2", debug=False, num_devices=NCORES)

    f8h, f8l = dt.float8e4, dt.float8e5
    # X and Wq/Wk/Wv come pre-split into an e4m3 hi plane and an e5m2 lo
    # (residual) plane, pre-interleaved for DoubleRow: [p, g, i, n] holds
    # element (256*g + 128*i + p, n).
    x8d = {
        "h": nc.dram_tensor("X8H", [128, D // 256, 2, t_len], f8h,
                            kind="ExternalInput"),
        "l": nc.dram_tensor("X8L", [128, D // 256, 2, t_len], f8l,
                            kind="ExternalInput"),
    }
    w8d = {
        (nm, pl): nc.dram_tensor(f"W8{nm.upper()}{pl.upper()}",
                                 [128, D // 256, 2, GC],
                                 f8h if pl == "h" else f8l,
                                 kind="ExternalInput")
        for nm in ("q", "k", "v") for pl in ("h", "l")
    }
    wo = nc.dram_tensor("WO", [GC, D], f16, kind="ExternalInput")
    ot = nc.dram_tensor("OT", [D, t_len], f16, kind="ExternalOutput")

    with tile.TileContext(nc) as tc:
        with (
            tc.tile_pool(name="w", bufs=1) as wpool,
            tc.tile_pool(name="proj", bufs=1) as proj,
            tc.tile_pool(name="pt", bufs=1) as ptpool,
            tc.tile_pool(name="small", bufs=4) as small,
            tc.tile_pool(name="ot", bufs=8) as otpool,
            tc.tile_pool(name="ps", bufs=2, space="PSUM") as pspool,
            tc.tile_pool(name="st", bufs=2, space="PSUM") as stpool,
            tc.tile_pool(name="ct", bufs=2, space="PSUM") as ctpool,
        ):
            # ---- load inputs (weights first so matmuls can start early) ----
            ng = D // 256  # DoubleRow contraction groups (4)
            w_sb = {}
            for nm in ("q", "k", "v"):
                for pl in ("h", "l"):
                    t_ = wpool.tile([128, ng, 2, GC],
                                    f8h if pl == "h" else f8l,
                                    tag=f"w{nm}{pl}", name=f"w{nm}{pl}")
                    w_sb[(nm, pl)] = t_
            x_sb = {
                pl: wpool.tile([128, ng, 2, t_len],
                               f8h if pl == "h" else f8l,
                               tag=f"x{pl}", name=f"x{pl}")
                for pl in ("h", "l")
            }
            half = t_len // 2

            def load_x(pl, g, lo, eng):
                sl = slice(0, half) if lo else slice(half, t_len)
                eng.dma_start(out=x_sb[pl][:, g, :, sl],
                              in_=x8d[pl].ap()[:, g, :, sl])

            # startup stream split across both HWDGE queues; hi planes and
            # first token-halves land early so projection chains can start
            # while the rest streams in.
            nc.sync.dma_start(out=w_sb[("q", "h")], in_=w8d[("q", "h")].ap())
            nc.scalar.dma_start(out=w_sb[("k", "h")], in_=w8d[("k", "h")].ap())
            for g in range(ng):
                load_x("h", g, True, nc.sync if g % 2 == 0 else nc.scalar)
            nc.sync.dma_start(out=w_sb[("q", "l")], in_=w8d[("q", "l")].ap())
            nc.scalar.dma_start(out=w_sb[("k", "l")], in_=w8d[("k", "l")].ap())
            for g in range(ng):
                load_x("l", g, True, nc.sync if g % 2 == 1 else nc.scalar)
            nc.sync.dma_start(out=w_sb[("v", "h")], in_=w8d[("v", "h")].ap())
            nc.scalar.dma_start(out=w_sb[("v", "l")], in_=w8d[("v", "l")].ap())
            for g in range(ng):
                load_x("h", g, False, nc.sync if g % 2 == 0 else nc.scalar)
            for g in range(ng):
                load_x("l", g, False, nc.sync if g % 2 == 1 else nc.scalar)
            wo_sb = wpool.tile([128, 2, D], f16, tag="wo")
            nc.scalar.dma_start(
                out=wo_sb, in_=wo.ap().rearrange("(c p) n -> p c n", p=128)
            )

            qt_sb = [proj.tile([128, t_len], f16, tag=f"qt{m}", name=f"qt{m}")
                     for m in range(2)]
            kt_sb = [proj.tile([128, t_len], f16, tag=f"kt{m}", name=f"kt{m}")
                     for m in range(2)]
            v_sb = proj.tile([128, n_tt, HPC, HD + 1], f16, tag="v")
            nc.gpsimd.memset(v_sb, 1.0)
            ct_sb = [proj.tile([128, t_len], f16, tag=f"ct{m}", name=f"ctn{m}")
                     for m in range(2)]
            # pt tiles: one per (hp, kb) so a full q-tile's worth stays live
            pt_sb = [[ptpool.tile([128, 2, NQ], f16, tag=f"pt{hp}_{kb}",
                                  name=f"pt{hp}_{kb}")
                      for kb in range(n_tt)] for hp in range(2)]
            # DELTA-DEBUG step 4: TWO CONSECUTIVE fp8 tiles (hp0, kb0+kb1,
            # qt>=1) — tests the back-to-back ACT fp8 exp write hypothesis
            pt8_sb = [ptpool.tile([128, 2, NQ], dt.float8e4,
                                  tag=f"pt8_{kb}", name=f"pt8_{kb}")
                      for kb in range(2)]

            def pt_for(qt, hp, kb):
                if hp == 0 and kb < 2 and kb < 4 * qt:
                    return pt8_sb[kb]
                return pt_sb[hp][kb]
            # unnormalized C^T staging [q, (head,qb), hd+1] per hp
            ctu_sb = [proj.tile([128, 2 * (NQ // QB), HD + 1], f16,
                                tag=f"ctu{hp}", name=f"ctu{hp}")
                      for hp in range(2)]
            linv_sb = [proj.tile([128, 2 * (NQ // QB)], f32, tag=f"linv{hp}",
                                 name=f"linv{hp}") for hp in range(2)]

            diag_mask = proj.tile([128, 2, NQ], f16, tag="dmask")
            nc.vector.memset(diag_mask, 1.0)
            nc.gpsimd.affine_select(
                out=diag_mask,
                in_=diag_mask,
                compare_op=mybir.AluOpType.is_ge,
                fill=0.0,
                base=0,
                pattern=[[0, 2], [1, NQ]],
                channel_multiplier=-1,
            )
            expbias = proj.tile([128, 1], f32, tag="expbias")
            nc.vector.memset(expbias, -2.0)
            ident = proj.tile([128, 128], f16, tag="ident")
            from concourse.masks import make_identity
            make_identity(nc, ident)

            TERMS = (("h", "h"), ("h", "l"), ("l", "h"))
            DR = mybir.MatmulPerfMode.DoubleRow

            def qk_chain(dst, wname, m, n, hf, box):
                # hi/lo fp8 DoubleRow: X@W = XhWh + XhWl + XlWh (each term
                # 256-deep per group). Emitted as two half-chains (separate
                # work-queue pops so fillers fit the exp-stream slack) that
                # share one ps tile as sequential accumulation groups.
                if hf == 0:
                    box["ps"] = pspool.tile([128, NQ], f32, tag="ps",
                                            name="ps")
                ps = box["ps"]
                c0 = n * NQ + hf * 256
                for ti, (xp, wp) in enumerate(TERMS):
                    for g in range(ng):
                        nc.tensor.matmul(
                            ps[:, hf * 256:(hf + 1) * 256],
                            w_sb[(wname, wp)][:, g, :,
                                              m * 128:(m + 1) * 128],
                            x_sb[xp][:, g, :, c0:c0 + 256],
                            start=(ti == 0 and g == 0),
                            stop=(ti == 2 and g == ng - 1),
                            perf_mode=DR,
                        )
                if hf == 1:
                    nc.vector.tensor_copy(
                        dst[m][:, n * NQ:(n + 1) * NQ], ps)

            def v_chain(tt):
                ps = pspool.tile([128, NQ], f32, tag="ps", name="ps")
                t0 = tt * 128
                for ti, (xp, wp) in enumerate(TERMS):
                    for g in range(ng):
                        nc.tensor.matmul(
                            ps[:, 0:GC],
                            x_sb[xp][:, g, :, t0:t0 + 128],
                            w_sb[("v", wp)][:, g, :, :],
                            start=(ti == 0 and g == 0),
                            stop=(ti == 2 and g == ng - 1),
                            perf_mode=DR,
                        )
                nc.vector.tensor_copy(
                    v_sb[:, tt, :, 0:HD],
                    ps[:, 0:GC].rearrange("p (h d) -> p h d", h=HPC),
                )  # PSUM source: must be DVE (GPSIMD cannot access PSUM)

            def proj_slice_chains(n):
                # m-tile m feeds head-pair hp==m: label per (n, m) so each
                # A(n, hp) only drains its own half of the qk chains
                chains = []
                for m in range(2):
                    for dst, wname in ((qt_sb, "q"), (kt_sb, "k")):
                        box = {}
                        for hf in range(2):
                            chains.append(
                                (("qk", n),
                                 lambda d=dst, w=wname, mm=m, h=hf, bx=box:
                                 qk_chain(d, w, mm, n, h, bx))
                            )
                for tt in range(4 * n, 4 * n + 4):
                    chains.append((("v", n), lambda t=tt: v_chain(t)))
                return chains

            def attn_A(qt, hp, step=None):
                """S matmuls + exp into persistent pt tiles."""
                q0 = qt * NQ
                nkb = (q0 + NQ) // KB
                for kb in range(nkb):
                    if step is not None:
                        step()
                    k0 = kb * KB
                    off = max(0, k0 - q0)   # fully-masked q prefix
                    w = NQ - off
                    diag = k0 + KB > q0
                    st = stpool.tile([128, 2 * NQ], f32, tag="st", name="st")
                    st3 = st.rearrange("p (i q) -> p i q", i=2)
                    for i in range(2):
                        nc.tensor.matmul(
                            st3[:, i, off:],
                            kt_sb[hp][i * 64:(i + 1) * 64, k0:k0 + KB],
                            qt_sb[hp][i * 64:(i + 1) * 64,
                                      q0 + off:q0 + NQ],
                            start=True,
                            stop=True,
                            tile_position=(64 * i, 0),
                        )
                    pt3 = pt_for(qt, hp, kb)
                    # bias -2: keeps exp within fp8e4 range (max 240);
                    # the uniform e^-2 factor cancels in the l-normalization
                    nc.scalar.activation(
                        out=pt3[:, :, off:], in_=st3[:, :, off:],
                        func=mybir.ActivationFunctionType.Exp,
                        scale=float(1.0 / np.sqrt(HD)),
                        bias=expbias[:, 0:1],
                    )
                    if diag:  # zero where k > q (x < p in slice coords)
                        nc.vector.tensor_mul(
                            pt3[:, :, off:],
                            pt3[:, :, off:],
                            diag_mask[:, :, 0:w],
                        )

            def attn_B(qt, hp, step=None):
                """Transposed PV: accumulate CT_t[q, 65] per (head, qb),
                4 accs packed per PSUM bank, then normalize + transpose."""
                q0 = qt * NQ
                nqb = NQ // QB  # 4
                for quad in range(2):  # 2 quads of (i, qb) pairs per hp
                    if step is not None:
                        step()
                    acc = ctpool.tile([128, 4, HD + 1], f32, tag="ct",
                                      name="ct")
                    for slot in range(4):
                        idx = quad * 4 + slot
                        i, qb = divmod(idx, nqb)
                        last_kb = 4 * qt + qb
                        for kb in range(last_kb + 1):
                            nc.tensor.matmul(
                                acc[:, slot, :],
                                pt_for(qt, hp, kb)[:, i,
                                                   qb * QB:(qb + 1) * QB],
                                v_sb[:, kb, 2 * hp + i, :],
                                start=(kb == 0),
                                stop=(kb == last_kb),
                            )
                    # drain quad to SBUF (unnormalized, incl l column) and
                    # take reciprocals now: the DVE chain for quad 0 runs
                    # while the PE is still on quad 1's accumulation.
                    nc.vector.tensor_copy(
                        ctu_sb[hp][:, quad * 4:quad * 4 + 4, :], acc
                    )
                    nc.vector.reciprocal(
                        out=linv_sb[hp][:, quad * 4:quad * 4 + 4],
                        in_=ctu_sb[hp][:, quad * 4:quad * 4 + 4, HD],
                    )
                # normalize + transpose back per (head, qb)
                for i in range(2):
                    tr = ctpool.tile([64, nqb, QB], f16, tag="ct", name="tr")
                    for qb in range(nqb):
                        idx = i * nqb + qb
                        ctn = small.tile([128, HD], f16, tag="ctn",
                                         name="ctn")
                        nc.vector.tensor_scalar_mul(
                            ctn, ctu_sb[hp][:, idx, 0:HD],
                            linv_sb[hp][:, idx:idx + 1],
                        )
                        nc.tensor.transpose(tr[:, qb, :], ctn, ident)
                    nc.vector.tensor_copy(
                        ct_sb[hp][i * 64:(i + 1) * 64, q0:q0 + NQ],
                        tr.rearrange("p a b -> p (a b)"),
                    )

            def o_chain(m, n, use_st=False):
                if use_st:  # tail: borrow idle st banks as extra ps buffers
                    ps = stpool.tile([128, 2 * NQ], f32, tag="st",
                                     name="st")[:, 0:NQ]
                else:
                    ps = pspool.tile([128, NQ], f32, tag="ps", name="ps")
                for cc in range(2):
                    nc.tensor.matmul(
                        ps,
                        wo_sb[:, cc, m * 128:(m + 1) * 128],
                        ct_sb[cc][:, n * NQ:(n + 1) * NQ],
                        start=(cc == 0),
                        stop=(cc == 1),
                    )
                o_sb = otpool.tile([128, NQ], f16, name="o_sb")
                nc.vector.tensor_copy(o_sb, ps)
                nc.sync.dma_start(
                    out=ot.ap()[m * 128:(m + 1) * 128,
                                n * NQ:(n + 1) * NQ],
                    in_=o_sb,
                )

            def out_proj_chains(n, use_st=False):
                return [(("o", n), lambda m=m: o_chain(m, n, use_st and
                                                       m % 2 == 1))
                        for m in range(n_kc)]

            def proj_first_qk(step=None):
                """QT/KT n=0 via the idle st-pool banks: 4 accumulation
                groups in flight so the PE tracks XT chunk arrivals."""
                st_a = stpool.tile([128, 2 * NQ], f32, tag="st", name="st")
                st_b = stpool.tile([128, 2 * NQ], f32, tag="st", name="st")
                regions = [
                    (qt_sb, "q", 0, st_a[:, 0:NQ]),
                    (qt_sb, "q", 1, st_a[:, NQ:2 * NQ]),
                    (kt_sb, "k", 0, st_b[:, 0:NQ]),
                    (kt_sb, "k", 1, st_b[:, NQ:2 * NQ]),
                ]
                for kc in range(n_kc):
                    for dst, wname, m, reg in regions:
                        nc.tensor.matmul(
                            reg,
                            w_sb[wname][:, kc, m * 128:(m + 1) * 128],
                            xt_sb[kc][:, 0:NQ],
                            start=(kc == 0),
                            stop=(kc == n_kc - 1),
                        )
                    if step is not None and kc % 2 == 1:
                        step()
                for dst, wname, m, reg in regions:
                    nc.vector.tensor_copy(dst[m][:, 0:NQ], reg)

            # ---- pipelined schedule ----
            # PE-heavy proj/out-proj chains are interleaved into the
            # ACT-bound A phases via a labeled work queue. Deps are by
            # emission order, so chains must be emitted before consumers:
            # ("qk", n) before attn_A(n); ("v", n) before attn_B(n).
            pending = []

            def step():
                if pending:
                    pending.pop(0)[1]()

            def drain(label):
                while any(lab == label for lab, _ in pending):
                    step()

            # Software-pipelined rotation: B(qt) blocks sit between A(qt+1)
            # phases so the PE has big blocks of ready work while ACT works
            # through the exp stream of the surrounding A phases.
            pending += proj_slice_chains(0)
            pending += proj_slice_chains(1)
            drain(("qk", 0))
            attn_A(0, 0, step)
            attn_A(0, 1, step)
            for qt in range(n_qt):
                drain(("v", qt))
                attn_B(qt, 0, step)
                if qt + 1 < n_qt:
                    drain(("qk", qt + 1))
                    attn_A(qt + 1, 0, step)
                attn_B(qt, 1, step)
                if qt + 1 < n_qt:
                    attn_A(qt + 1, 1, step)
                    if qt + 2 < n_qt:
                        pending += proj_slice_chains(qt + 2)
                pending += out_proj_chains(qt, use_st=(qt == n_qt - 1))
            while pending:
                step()

    nc.compile()
    return nc


def get_nc(t_len=T):
    if t_len not in _cache:
        _cache[t_len] = _build(t_len)
    return _cache[t_len]


def _split8(a):
    """Split [D, N] into an e4m3 hi plane + e5m2 residual plane, both in
    the DoubleRow layout [128, D//256, 2, N]."""
    import ml_dtypes
    hi = a.astype(ml_dtypes.float8_e4m3)
    lo = (a - hi.astype(np.float32)).astype(ml_dtypes.float8_e5m2)
    d, n = a.shape

    def lay(p):
        return np.ascontiguousarray(
            p.reshape(d // 256, 2, 128, n).transpose(2, 0, 1, 3))

    return lay(hi), lay(lo)


def make_in_maps(X, Wq, Wk, Wv, Wo):
    X = np.asarray(X, dtype=np.float32)
    Wq = np.asarray(Wq, dtype=np.float32)
    Wk = np.asarray(Wk, dtype=np.float32)
    Wv = np.asarray(Wv, dtype=np.float32)
    Wo = np.asarray(Wo, dtype=np.float32)
    in_maps = []
    for c in range(NCORES):
        b, g = divmod(c, 4)
        cols = slice(g * GC, (g + 1) * GC)
        xh, xl = _split8(np.ascontiguousarray(X[b].T))
        m = {"X8H": xh, "X8L": xl,
             "WO": np.ascontiguousarray(Wo[cols, :]).astype(np.float16)}
        for nm, w in (("q", Wq), ("k", Wk), ("v", Wv)):
            wh, wl = _split8(np.ascontiguousarray(w[:, cols]))
            m[f"W8{nm.upper()}H"] = wh
            m[f"W8{nm.upper()}L"] = wl
        in_maps.append(m)
    return in_maps


def gather_out(results, bo):
    out = np.zeros((B, T, D), dtype=np.float32)
    for c in range(NCORES):
        out[c // 4] += results[c]["OT"].T.astype(np.float32)
    out += np.asarray(bo, dtype=np.float32)
    return out


def kernel(X, Wq, Wk, Wv, Wo, bo):
    from concourse import bass_utils

    nc = get_nc(T)
    in_maps = make_in_maps(X, Wq, Wk, Wv, Wo)
    res = bass_utils.run_bass_kernel_spmd(
        nc, in_maps, core_ids=list(range(NCORES))
    )
    return gather_out(res.results, bo)
